# revision 100
# baseline (speedup 1.0000x reference)
"""Cross-encoding kernel for Trainium2 (Bass/Tile), 8-core batch-parallel.

Per batch b:
    query = Q W1 + b1 ; key = A W2 + b2
    S = query key^T / sqrt(d)
    eq = softmax_rows(S) @ A          (qk attention)
    ea = softmax_cols(S)^T @ Q        (kq attention)

Host folds the projections: S_full/sqrt(d) = (Q M) A^T / sqrt(d) + u 1^T
+ 1 v^T with M = W1 W2^T, u = Q W1 b2 / sqrt(d), v = A W2 b1 / sqrt(d)
(the b1.b2 constant cancels in both softmaxes). The qm = Q M product is
computed on the host in f32 and shipped as an fp8 input (the PE would
have recomputed it from bf16 at worse precision).

On device (per batch, all PE operands bf16/fp8):
  ph1: for each a-stripe of 256: scores S[q,a] once (fp8 DoubleRow);
       ACT exp with per-partition bias u -> G = exp(s+u).
       EA AV matmuls + EA-denominator ones-matmuls consume G directly
       (softmax column-normalize cancels exp(u) row factors), so the PE
       path is S -> exp -> EA with no DVE hop. DVE separately forms
       E = G * exp(v) (needed for the EQ path), its free-axis reduce
       accumulates the EQ denominators, and a DMA xbar transpose moves
       E into the resident E^T buffer off the PE.
  ph2: EQ AV matmuls from E^T x A_nat, normalized by the accumulated
       EQ denominators.

The score matrix is computed ONCE; EA denominators accumulate on the PE
(1-wide ones-column matmuls sharing the EA stationary tensors) so the
stripe-end normalize never waits on the transposes.
"""
import math

import numpy as np

B, LQ, LA, D = 16, 2048, 2048, 1024
NCORES = 8
BPC = B // NCORES

_cached = {}


def _build(lq=LQ, la=LA, d=D, bpc=BPC):
    import concourse.bass as bass
    import concourse.tile as tile
    from concourse import bacc, mybir

    f32 = mybir.dt.float32
    bf16 = mybir.dt.bfloat16
    fp8 = mybir.dt.float8e4
    DR = mybir.MatmulPerfMode.DoubleRow
    ec_n = d // 128          # 8 contraction chunks
    nqt, nat = lq // 128, la // 128   # 16, 16
    nga = la // 256          # 8 a-stripes
    ngq = lq // 256          # 8 q-groups (phase 2)
    inv_sqrt_d = 1.0 / math.sqrt(d)

    nc = bacc.Bacc("TRN2", target_bir_lowering=False, debug=False)

    # all operand layouts are partition-major (and q/a-block-major for the
    # score operands) so every input DMA is one contiguous descriptor per
    # partition
    qm_in = nc.dram_tensor("qm_in", [bpc, 4, 128, ec_n, lq // 4], fp8, kind="ExternalInput").ap()
    at_in = nc.dram_tensor("at_in", [bpc, 4, 128, ec_n, la // 4], fp8, kind="ExternalInput").ap()
    qn_in = nc.dram_tensor("qn_in", [bpc, 128, nqt, d], bf16, kind="ExternalInput").ap()
    an_in = nc.dram_tensor("an_in", [bpc, 128, nat, d], bf16, kind="ExternalInput").ap()
    ones_in = nc.dram_tensor("ones_in", [128, 1], bf16, kind="ExternalInput").ap()
    ub_in = nc.dram_tensor("ub_in", [bpc, lq], f32, kind="ExternalInput").ap()
    ev_in = nc.dram_tensor("ev_in", [bpc, 128, la], bf16, kind="ExternalInput").ap()
    eq_out = nc.dram_tensor("eq_out", [bpc, nqt, 128, d], bf16, kind="ExternalOutput").ap()
    ea_out = nc.dram_tensor("ea_out", [bpc, nat, 128, d], bf16, kind="ExternalOutput").ap()

    Exp = mybir.ActivationFunctionType.Exp
    Copy = mybir.ActivationFunctionType.Copy
    AX = mybir.AxisListType.X
    ADD = mybir.AluOpType.add
    MULT = mybir.AluOpType.mult

    with tile.TileContext(nc) as tc:
        with (
            tc.tile_pool(name="const", bufs=1) as constp,
            tc.tile_pool(name="qmp", bufs=1) as qmp,
            tc.tile_pool(name="etp", bufs=1) as etp,
            tc.tile_pool(name="natp", bufs=2) as natp,
            tc.tile_pool(name="ats", bufs=1) as ats,
            tc.tile_pool(name="ep", bufs=6) as ep,
            tc.tile_pool(name="epf", bufs=10) as epf,
            tc.tile_pool(name="stg", bufs=4) as stg,
            tc.tile_pool(name="small", bufs=1) as small,
            tc.tile_pool(name="psO", bufs=2, space=bass.MemorySpace.PSUM) as psO,
            tc.tile_pool(name="psS", bufs=2, space=bass.MemorySpace.PSUM) as psS,
            tc.tile_pool(name="psDen", bufs=2, space=bass.MemorySpace.PSUM) as psDen,
        ):
            ones_sb = constp.tile([128, 1], bf16, tag="ones")
            nc.sync.dma_start(out=ones_sb, in_=ones_in)

            def load_front(bi):
                """Dispatch batch bi's ph1 inputs. Interleave qm/at/qn
                blocks so stripe 0's score and EA matmuls see their first
                operands as early as possible."""
                qm_sb = qmp.tile([128, 4, ec_n, lq // 4], fp8, tag="qm", name="qm_sb")
                at_full = ats.tile([128, 4, ec_n, la // 4], fp8, tag="at", name="at_full")
                qn_sb = natp.tile([128, nqt, d], bf16, tag="nat", name="qn_sb")
                # the first score operands dispatch before anything else
                # (they gate the very first matmul); stripe 0 then walks
                # the qm q-blocks and qn chunk by chunk; at block k is
                # first needed at stripe 2k
                nc.sync.dma_start(out=qm_sb[:, 0], in_=qm_in[bi, 0])
                nc.sync.dma_start(out=at_full[:, 0], in_=at_in[bi, 0])
                ub_sb = small.tile([128, nqt], f32, tag="ub")
                nc.sync.dma_start(out=ub_sb, in_=ub_in[bi].rearrange("(t p) -> p t", p=128))
                for blk in range(4):
                    if blk >= 1:
                        nc.sync.dma_start(out=qm_sb[:, blk], in_=qm_in[bi, blk])
                        nc.sync.dma_start(out=at_full[:, blk], in_=at_in[bi, blk])
                    nc.sync.dma_start(
                        out=qn_sb[:, blk * 4:(blk + 1) * 4, :],
                        in_=qn_in[bi][:, blk * 4:(blk + 1) * 4, :])
                ev_sb = small.tile([128, la], bf16, tag="ev")
                nc.sync.dma_start(out=ev_sb, in_=ev_in[bi])
                return ub_sb, qm_sb, at_full, qn_sb, ev_sb

            # warm up the PE during the otherwise-idle initial load window:
            # the tensor engine needs ~3us of continuous execution to ramp
            # from its low power-state clock to 2.4GHz, so without this the
            # first stripe runs at reduced frequency. 1-wide matmuls on the
            # resident ones vector burn ~5us of decode/issue at zero cost.
            front = load_front(0)
            warm = psS.tile([128, 256], f32, tag="s", name="warm")
            for _ in range(100):
                nc.tensor.matmul(warm[0:1, 0:1], ones_sb, ones_sb,
                                 start=True, stop=True)
            for bi in range(bpc):
                ub_sb, qm_sb, at_full, qn_sb, ev_sb = front
                eqd_part = small.tile([128, nqt, nga], f32, tag="eqd")

                # an for ph2 (EQ AV rhs); loads during ph1
                an_sb = natp.tile([128, nat, d], bf16, tag="nat", name="an_sb")
                for blk in range(4):
                    nc.sync.dma_start(
                        out=an_sb[:, blk * 4:(blk + 1) * 4, :],
                        in_=an_in[bi][:, blk * 4:(blk + 1) * 4, :])

                # E^T buffer split into per-qg q-column tiles: ph2's EQ
                # group qg reads only ets[qg], so the next batch's
                # transposes into ets[k] unblock as soon as EQ group k is
                # done instead of waiting for the whole phase
                ets = [etp.tile([128, nat, 256], bf16, tag=f"et{k}", name=f"et{k}")
                       for k in range(ngq)]

                # ---- ph1: stripes over a ----
                for g in range(nga):
                    pacc = [psO.tile([128, d], f32, tag="acc", name="pacc")
                            for _ in range(2)]
                    din = [psDen.tile([128, 1], f32, tag="din", name="din")
                           for _ in range(2)]

                    def consume(e_t, efull, ch, g=g, pacc=pacc, din=din):
                        h = (ch % 2) * 128
                        nc.sync.dma_start_transpose(
                            out=ets[ch // 2][:, 2 * g:2 * g + 2, h:h + 128],
                            in_=efull)
                        for t2 in range(2):
                            for dh in range(2):
                                nc.tensor.matmul(
                                    pacc[t2][:, dh * 512:(dh + 1) * 512],
                                    e_t[:, t2 * 128:(t2 + 1) * 128],
                                    qn_sb[:, ch, dh * 512:(dh + 1) * 512],
                                    start=(ch == 0), stop=(ch == nqt - 1))
                            nc.tensor.matmul(
                                din[t2],
                                e_t[:, t2 * 128:(t2 + 1) * 128],
                                ones_sb,
                                start=(ch == 0), stop=(ch == nqt - 1))

                    # consume() lags the producer chain by two chunks so
                    # the PE has S-matmul work queued while the previous
                    # stripe's normalize frees the pacc PSUM slots
                    pending = []
                    for ch in range(nqt):
                        ps = psS.tile([128, 256], f32, tag="s", name="ps")
                        cb, cr = ch // 4, (ch % 4) * 128
                        gb, gr = g // 2, (g % 2) * 256
                        for ej in range(ec_n // 2):
                            nc.tensor.matmul(
                                ps,
                                qm_sb[:, cb, 2 * ej:2 * ej + 2, cr:cr + 128],
                                at_full[:, gb, 2 * ej:2 * ej + 2, gr:gr + 256],
                                start=(ej == 0), stop=(ej == ec_n // 2 - 1),
                                perf_mode=DR)
                        e_t = ep.tile([128, 256], bf16, tag="e", name="e_t")
                        nc.scalar.activation(
                            out=e_t, in_=ps, func=Exp, scale=inv_sqrt_d / 512.0,
                            bias=ub_sb[:, ch:ch + 1])
                        # efull has its own deep ring: its only reader is
                        # the DMA transpose, whose lag would otherwise
                        # transitively stall the exp stream (transpose ->
                        # efull ring -> DVE mul -> e_t ring -> exp -> PE)
                        efull = epf.tile([128, 256], bf16, tag="ef", name="efull")
                        nc.vector.tensor_mul(
                            efull, e_t, ev_sb[:, g * 256:(g + 1) * 256])
                        nc.vector.tensor_reduce(
                            out=eqd_part[:, ch, g:g + 1], in_=efull,
                            axis=AX, op=ADD)
                        pending.append((e_t, efull, ch))
                        if len(pending) > 4:
                            consume(*pending.pop(0))
                    for p in pending:
                        consume(*p)

                    # EA normalize straight from PSUM (pacc + din are both
                    # PE outputs, available together at stripe end); each
                    # store is dispatched from the queue that produced st,
                    # so no queue blocks at its head waiting cross-engine
                    for t2 in range(2):
                        rcp = ep.tile([128, 1], f32, tag="rcpa", name="rcp")
                        nc.vector.reciprocal(out=rcp, in_=din[t2])
                        st = stg.tile([128, d], bf16, tag="st", name="st")
                        if t2 == 0:
                            nc.scalar.activation(
                                out=st, in_=pacc[t2], func=Copy, scale=rcp)
                            nc.scalar.dma_start(out=ea_out[bi, 2 * g + t2], in_=st)
                        else:
                            nc.vector.tensor_scalar_mul(
                                out=st, in0=pacc[t2], scalar1=rcp)
                            nc.sync.dma_start(out=ea_out[bi, 2 * g + t2], in_=st)

                # prefetch next batch's ph1 inputs while ph2 runs
                if bi + 1 < bpc:
                    front = load_front(bi + 1)

                # EQ denominators: accumulate stripe partials, reciprocal
                eqd = small.tile([128, nqt], f32, tag="eqs")
                nc.vector.tensor_reduce(out=eqd, in_=eqd_part, axis=AX, op=ADD)
                eqr = small.tile([128, nqt], f32, tag="eqr")
                nc.vector.reciprocal(out=eqr, in_=eqd)

                # ---- ph2: EQ AV from E^T ----
                for qg in range(ngq):
                    qacc = [psO.tile([128, d], f32, tag="acc", name="qacc")
                            for _ in range(2)]
                    # t2 outer: qacc[0]'s accumulation group closes halfway
                    # through the qg, so its normalize overlaps the second
                    # half and the next qg's first matmul never waits
                    for t2 in range(2):
                        for ac in range(nat):
                            for dh in range(2):
                                nc.tensor.matmul(
                                    qacc[t2][:, dh * 512:(dh + 1) * 512],
                                    ets[qg][:, ac, t2 * 128:(t2 + 1) * 128],
                                    an_sb[:, ac, dh * 512:(dh + 1) * 512],
                                    start=(ac == 0), stop=(ac == nat - 1))
                    for t2 in range(2):
                        st = stg.tile([128, d], bf16, tag="st", name="st2")
                        if t2 == 0:
                            nc.vector.tensor_scalar_mul(
                                out=st, in0=qacc[t2],
                                scalar1=eqr[:, 2 * qg + t2: 2 * qg + t2 + 1])
                            nc.sync.dma_start(out=eq_out[bi, 2 * qg + t2], in_=st)
                        else:
                            nc.scalar.activation(
                                out=st, in_=qacc[t2], func=Copy,
                                scale=eqr[:, 2 * qg + t2: 2 * qg + t2 + 1])
                            nc.scalar.dma_start(out=eq_out[bi, 2 * qg + t2], in_=st)

    nc.compile()
    return nc


def _get_nc():
    if "nc" not in _cached:
        _cached["nc"] = _build()
    return _cached["nc"]


def _bf16():
    import ml_dtypes
    return ml_dtypes.bfloat16


def _fp8():
    import ml_dtypes
    return ml_dtypes.float8_e4m3fn


def _pack_inputs(Qc, Ac, M, w1b2, w2b1, lq, la, d):
    bf16 = _bf16()
    ec_n = d // 128
    bpc = Qc.shape[0]
    inv = 1.0 / math.sqrt(d)
    ub = (Qc.astype(np.float64) @ w1b2 * inv).astype(np.float32)     # [bpc, lq]
    v = (Ac.astype(np.float64) @ w2b1 * inv).astype(np.float32)      # [bpc, la]
    ev = np.exp(v).astype(bf16)
    ev_rep = np.broadcast_to(ev[:, None, :], (bpc, 128, la)).copy()
    # qm = Q M in f32 on host (the PE used to do this from bf16), fp8 x32
    qm = np.matmul(Qc, M) * 32.0                                     # [bpc, lq, d]

    def _blockmajor(x):  # [bpc, L, d] fp8 -> [bpc, 4, 128, ec_n, L//4]
        L = x.shape[1]
        return np.ascontiguousarray(
            x.reshape(bpc, 4, L // 4, ec_n, 128).transpose(0, 1, 4, 3, 2))

    def _partmajor(x):   # [bpc, L, d] bf16 -> [bpc, 128, L//128, d]
        L = x.shape[1]
        return np.ascontiguousarray(
            x.reshape(bpc, L // 128, 128, d).transpose(0, 2, 1, 3))

    return {
        "qm_in": _blockmajor(qm.astype(_fp8())),
        "at_in": _blockmajor((Ac * 16.0).astype(_fp8())),
        "qn_in": _partmajor(Qc.astype(bf16)),
        "an_in": _partmajor(Ac.astype(bf16)),
        "ub_in": ub,
        "ev_in": ev_rep,
    }


def _fold_weights(W1, b1, W2, b2, d):
    M = (W1.astype(np.float64) @ W2.astype(np.float64).T).astype(np.float32)
    w1b2 = W1.astype(np.float64) @ b2.astype(np.float64)
    w2b1 = W2.astype(np.float64) @ b1.astype(np.float64)
    return M, w1b2, w2b1


def _reference_fallback(Q, A, mask, W1, b1, W2, b2):
    NEG = np.float32(-1e9)
    eqs, eas = [], []
    for b in range(Q.shape[0]):
        query = Q[b] @ W1 + b1
        key = A[b] @ W2 + b2
        s = (query @ key.T) / np.float32(math.sqrt(Q.shape[-1]))
        s = np.where(mask[b] == 0, NEG, s).astype(np.float32)
        sq = s - s.max(axis=1, keepdims=True)
        eq_w = np.exp(sq); eq_w /= eq_w.sum(axis=1, keepdims=True)
        sa = s.T - s.T.max(axis=1, keepdims=True)
        ea_w = np.exp(sa); ea_w /= ea_w.sum(axis=1, keepdims=True)
        eqs.append(eq_w @ A[b])
        eas.append(ea_w @ Q[b])
    return np.stack(eqs), np.stack(eas)


def kernel(Q, A, mask, W1, b1, W2, b2):
    Q = np.ascontiguousarray(Q, dtype=np.float32)
    A = np.ascontiguousarray(A, dtype=np.float32)
    W1 = np.ascontiguousarray(W1, dtype=np.float32)
    W2 = np.ascontiguousarray(W2, dtype=np.float32)
    b1 = np.ascontiguousarray(b1, dtype=np.float32)
    b2 = np.ascontiguousarray(b2, dtype=np.float32)

    if not np.all(mask == 1):
        return _reference_fallback(Q, A, mask, W1, b1, W2, b2)

    from concourse import bass_utils

    nc = _get_nc()
    in_maps = _make_in_maps(Q, A, W1, b1, W2, b2)
    res = bass_utils.run_bass_kernel_spmd(nc, in_maps, core_ids=list(range(NCORES)))
    return _unpack(res.results)


def _make_in_maps(Q, A, W1, b1, W2, b2):
    bf16 = _bf16()
    M, w1b2, w2b1 = _fold_weights(W1, b1, W2, b2, D)
    in_maps = []
    for c in range(NCORES):
        sl = slice(c * BPC, (c + 1) * BPC)
        m = _pack_inputs(Q[sl], A[sl], M, w1b2, w2b1, LQ, LA, D)
        m.update({"ones_in": np.ones((128, 1), dtype=bf16)})
        in_maps.append(m)
    return in_maps


def _unpack(results):
    eq = np.empty((B, LQ, D), np.float32)
    ea = np.empty((B, LA, D), np.float32)
    for c in range(NCORES):
        out = results[c]
        eq[c * BPC:(c + 1) * BPC] = out["eq_out"].astype(np.float32).reshape(BPC, LQ, D)
        ea[c * BPC:(c + 1) * BPC] = out["ea_out"].astype(np.float32).reshape(BPC, LQ, D)
    return eq, ea
